# revision 9
# baseline (speedup 1.0000x reference)
"""CrossEntropyLossWithGaussianSmoothedLabels on 8 TRN2 NeuronCores.

Math: the reference's scatter-built smoothed label at class j is exactly
w[|j-t|] for |j-t|<=3 (w = [1, e^-.5, e^-1, e^-2]), clamped writes always
being overwritten by the nearer-distance write. So

  loss = mean_r( W_r * logsumexp(x_r) - sum_o w[|o|] * x_r[t_r+o] )

with W_r = sum of valid window weights. The gather term is computed on the
TensorEngine without any per-row gather:

  sum_r sum_o w[o] x[r, t_r+o] = sum_{|m-n|<=3} (H^T X)[m, n] * w[n-m]

where H is the one-hot target matrix, accumulated in PSUM via 6 banded
128x128 fp16 matmuls per 128-row tile (overlapping class blocks with
ownership-deduped band masks). logsumexp runs max-free (|x| < 6) via the
ScalarEngine's fused exp+accumulate; each core ships per-row sumexp and the
band dot, and the host applies W_r * ln(sumexp) and the mean in float64
(the all-reduce step of the data-parallel sharding).

Pipeline design (v1 had every engine 50-60% busy and a 51.6us span; dual
DMA queues measured STRICTLY WORSE - ~240GB/s aggregate vs ~360 single):
  - One f32 x stream on the sync HWDGE queue, one DMA per 128-row tile,
    x fully SBUF-resident so no ring-recycling semaphores exist.
  - The f32->f16 cast for the PE is split: gpsimd copies even tiles, DVE
    copies odd tiles (DVE also builds the one-hot rows), so no engine
    exceeds ~23us of work against the ~32us stream.
  - ACT warms Exp+Ln from the combined table set at t~0 via scale=0
    activations (no input dependency), streams 32 exp+accum tiles, and
    issues the final output DMA itself.
  - One-hot H rows and the iota row come from host-side index bookkeeping;
    h lives in a small ring recycled against PE progress.
"""

import math
from contextlib import ExitStack

import numpy as np

import concourse.bacc as bacc
from concourse import mybir
from concourse.bass_utils import run_bass_kernel_spmd

P = 128
C = 722
NCORES = 8
ROWS = 16 * 2048
RPC = ROWS // NCORES  # 4096 rows per core
NT = RPC // P         # 32 row tiles per core
NB = 6
HR = 8                # h ring depth
BLK = [0, 124, 248, 372, 496, 594]  # even bases -> 4B-aligned fp16 slices
URANGES = [(0, 124), (124, 248), (248, 372), (372, 496), (496, 594), (594, 722)]
WDEC = [1.0, math.exp(-0.5), math.exp(-1.0), math.exp(-2.0)]

f32 = mybir.dt.float32
f16 = mybir.dt.float16

AF = mybir.ActivationFunctionType
OP = mybir.AluOpType


def _band_masks() -> np.ndarray:
    """[128, 6*128] f32: block-local band weights, each global band entry
    owned by exactly one block (by min(m,n) ownership range)."""
    m = np.zeros((P, NB * P), np.float32)
    for b in range(NB):
        s = BLK[b]
        lo, hi = URANGES[b]
        for i in range(P):
            for o in range(-3, 4):
                j = i + o
                if 0 <= j < P:
                    mg, ng = s + i, s + j
                    if mg < C and ng < C and lo <= min(mg, ng) < hi:
                        m[i, b * P + j] = WDEC[abs(o)]
    return m


def _combined_act_set(arch: str) -> int:
    """Index of the act-func set containing both Exp and Ln."""
    from concourse.hw_specs import get_activation_tables

    try:
        tables = list(get_activation_tables(arch).items())
    except Exception:
        return -1
    for i, (_, funcs) in enumerate(tables):
        if AF.Exp in funcs and AF.Ln in funcs:
            return i
    return -1


def _build(rpc: int):
    nt = rpc // P
    nc = bacc.Bacc(
        "TRN2", target_bir_lowering=False, debug=False, num_devices=NCORES
    )

    xt = nc.dram_tensor("xt", [P, nt * C], f32, kind="ExternalInput").ap()
    tf = nc.dram_tensor("tf", [P, nt], f32, kind="ExternalInput").ap()
    band = nc.dram_tensor("band", [P, NB * P], f32, kind="ExternalInput").ap()
    iota = nc.dram_tensor("iota16", [P, C], f16, kind="ExternalInput").ap()
    out = nc.dram_tensor("out", [P, nt + 1], f32, kind="ExternalOutput").ap()

    x32 = nc.alloc_sbuf_tensor("x32", [P, nt, C], f32).ap()
    xh = nc.alloc_sbuf_tensor("xh", [P, nt, C], f16).ap()
    h_ring = nc.alloc_sbuf_tensor("h_ring", [P, HR, C], f16).ap()
    esc = nc.alloc_sbuf_tensor("esc", [P, C], f16).ap()
    outsb = nc.alloc_sbuf_tensor("outsb", [P, nt + 1], f32).ap()
    tf_sb = nc.alloc_sbuf_tensor("tf_sb", [P, nt], f32).ap()
    band_sb = nc.alloc_sbuf_tensor("band_sb", [P, NB * P], f32).ap()
    iota_sb = nc.alloc_sbuf_tensor("iota_sb", [P, C], f16).ap()
    warm = nc.alloc_sbuf_tensor("warm", [P, 1], f32).ap()
    warm_o = nc.alloc_sbuf_tensor("warm_o", [P, 1], f32).ap()
    mscr = nc.alloc_sbuf_tensor("mscr", [P, NB * P], f32).ap()

    psum = nc.alloc_psum_tensor("psumblk", [P, NB, 512], f32).ap()

    actset = _combined_act_set(nc.m.arch)

    with (
        nc.Block() as block,
        ExitStack() as _sems,
        nc.semaphore("t_sem") as t_sem,
        nc.semaphore("b_sem") as b_sem,
        nc.semaphore("i_sem") as i_sem,
        nc.semaphore("act_sem") as act_sem,
        nc.semaphore("bnd_sem") as bnd_sem,
        nc.semaphore("vcast") as vcast,
        nc.semaphore("gcast") as gcast,
        nc.semaphore("h_sem") as h_sem,
        nc.semaphore("pe_tile") as pe_tile,
        nc.semaphore("odma") as odma,
    ):
        xss = [_sems.enter_context(nc.semaphore(f"xss{i}")) for i in range(nt)]

        @block.sync
        def _(sync):
            for i in range(nt):
                sync.dma_start(
                    out=x32[:, i, :], in_=xt[:, i * C:(i + 1) * C]
                ).then_inc(xss[i], 16)
            sync.wait_ge(odma, 16)

        @block.gpsimd
        def _(gpsimd):
            # even-tile f32 -> f16 casts
            for i in range(0, nt, 2):
                gpsimd.wait_ge(xss[i], 16)
                gpsimd.tensor_copy(
                    out=xh[:, i, :], in_=x32[:, i, :]
                ).then_inc(gcast, 1)

        @block.scalar
        def _(scalar):
            if actset >= 0:
                scalar.add_instruction(
                    mybir.InstLoadActFuncSet(
                        name=nc.get_next_instruction_name(),
                        ins=[],
                        outs=[],
                        act_func_set_id=actset,
                    )
                )
            # scale=0 makes the input irrelevant: warms the tables with no
            # cross-engine dependency
            scalar.activation(out=warm_o, in_=warm, func=AF.Ln,
                              scale=0.0, bias=1.0)
            scalar.activation(out=warm_o, in_=warm, func=AF.Exp,
                              scale=0.0, bias=1.0)
            scalar.dma_start(out=tf_sb, in_=tf).then_inc(t_sem, 16)
            scalar.dma_start(out=iota_sb, in_=iota).then_inc(i_sem, 16)
            scalar.dma_start(out=band_sb, in_=band).then_inc(b_sem, 16)
            for i in range(nt):
                if i >= 2:
                    # act accumulator is pipelined 2-deep
                    scalar.wait_ge(act_sem, i - 1)
                scalar.wait_ge(xss[i], 16)
                scalar.activation(
                    out=esc, in_=x32[:, i, :], func=AF.Exp,
                    accum_out=outsb[:, i:i + 1],
                ).then_inc(act_sem, 1)
            scalar.wait_ge(act_sem, nt)
            scalar.wait_ge(bnd_sem, 1)
            scalar.dma_start(out=out, in_=outsb).then_inc(odma, 16)

        @block.vector
        def _(vector):
            vector.wait_ge(i_sem, 16)
            vector.wait_ge(t_sem, 16)
            for i in range(nt):
                if i % 2 == 1:
                    # odd-tile f32 -> f16 casts
                    vector.wait_ge(xss[i], 16)
                    vector.tensor_copy(
                        out=xh[:, i, :], in_=x32[:, i, :]
                    ).then_inc(vcast, 1)
                if i >= HR:
                    vector.wait_ge(pe_tile, i - HR + 1)
                vector.tensor_scalar(
                    out=h_ring[:, i % HR, :], in0=iota_sb,
                    scalar1=tf_sb[:, i:i + 1], scalar2=None,
                    op0=OP.is_equal, op1=OP.bypass,
                ).then_inc(h_sem, 1)
            vector.wait_ge(pe_tile, nt)
            vector.wait_ge(b_sem, 16)
            vector.scalar_tensor_tensor(
                out=mscr.rearrange("p (b n) -> p b n", b=NB),
                in0=psum[:, :, 0:P],
                scalar=0.0,
                in1=band_sb.rearrange("p (b n) -> p b n", b=NB),
                op0=OP.bypass,
                op1=OP.mult,
                accum_out=outsb[:, nt:nt + 1],
            ).then_inc(bnd_sem, 1)

        @block.tensor
        def _(pe):
            for i in range(nt):
                pe.wait_ge(h_sem, i + 1)
                if i % 2 == 0:
                    pe.wait_ge(gcast, i // 2 + 1)
                else:
                    pe.wait_ge(vcast, (i - 1) // 2 + 1)
                for b in range(NB):
                    s = BLK[b]
                    mm = pe.matmul(
                        psum[:, b, 0:P], h_ring[:, i % HR, s:s + P],
                        xh[:, i, s:s + P],
                        start=(i == 0), stop=(i == nt - 1),
                    )
                mm.then_inc(pe_tile, 1)

    nc.compile()
    return nc


def _shard_inputs(prediction: np.ndarray, target: np.ndarray, rpc: int, ncores: int):
    pred = np.asarray(prediction, dtype=np.float32).reshape(-1, C)
    tgt = np.asarray(target).reshape(-1).astype(np.int64)
    nt = rpc // P
    band = _band_masks()
    iota16 = np.broadcast_to(
        np.arange(C, dtype=np.float16)[None, :], (P, C)
    ).copy()
    in_maps = []
    for c in range(ncores):
        sl = slice(c * rpc, (c + 1) * rpc)
        # row i*128+p of the shard -> partition p, tile i
        xtc = np.ascontiguousarray(
            pred[sl].reshape(nt, P, C).transpose(1, 0, 2)
        ).reshape(P, nt * C)
        tfc = np.ascontiguousarray(
            tgt[sl].reshape(nt, P).T.astype(np.float32))
        in_maps.append({
            "xt": xtc,
            "tf": tfc,
            "band": band,
            "iota16": iota16,
        })
    return in_maps


def _host_combine(results, target: np.ndarray, nt: int) -> np.float32:
    tgt = np.asarray(target).reshape(-1).astype(np.int64)
    # W_r = 1 + sum_d w_d*([t>=d] + [t<=C-1-d])
    w_all = np.ones(tgt.shape, np.float64)
    for d in (1, 2, 3):
        w_all += WDEC[d] * (
            (tgt >= d).astype(np.float64) + (tgt <= C - 1 - d).astype(np.float64)
        )
    rpc = len(tgt) // len(results)
    tot = 0.0
    for ci, r in enumerate(results):
        o = np.asarray(r["out"], dtype=np.float64)
        wc = w_all[ci * rpc:(ci + 1) * rpc].reshape(nt, P).T
        tot += (wc * np.log(o[:, :nt])).sum() - o[:, nt:].sum()
    return np.float32(tot / len(tgt))


def kernel(prediction: np.ndarray, target: np.ndarray, _trace: bool = False):
    nc = _build(RPC)
    in_maps = _shard_inputs(prediction, target, RPC, NCORES)
    res = run_bass_kernel_spmd(
        nc, in_maps, core_ids=list(range(NCORES)), trace=_trace
    )
    loss = _host_combine(res.results, target, NT)
    if _trace:
        return loss, res
    return loss


# revision 10
# speedup vs baseline: 1.4650x; 1.4650x over previous
"""CrossEntropyLossWithGaussianSmoothedLabels on 8 TRN2 NeuronCores.

Math: the reference's scatter-built smoothed label at class j is exactly
w[|j-t|] for |j-t|<=3 (w = [1, e^-.5, e^-1, e^-2]), clamped writes always
being overwritten by the nearer-distance write. So

  loss = mean_r( W_r * logsumexp(x_r) - sum_o w[|o|] * x_r[t_r+o] )

with W_r = sum of valid window weights. The gather term is computed on the
TensorEngine without any per-row gather:

  sum_r sum_o w[o] x[r, t_r+o] = sum_{|m-n|<=3} (H^T X)[m, n] * w[n-m]

where H is the one-hot target matrix, accumulated in PSUM via 6 banded
128x128 fp16 matmuls per 128-row tile (overlapping class blocks with
ownership-deduped band masks). logsumexp runs max-free (|x| < 6) via the
ScalarEngine's fused exp+accumulate; each core ships per-row sumexp and the
band dot, and the host applies W_r * ln(sumexp) and the mean in float64
(the all-reduce step of the data-parallel sharding).

Pipeline design (v1 had every engine 50-60% busy and a 51.6us span; dual
DMA queues measured STRICTLY WORSE - ~240GB/s aggregate vs ~360 single):
  - One f32 x stream on the sync HWDGE queue, one DMA per 128-row tile,
    x fully SBUF-resident so no ring-recycling semaphores exist.
  - The f32->f16 cast for the PE is split: gpsimd copies even tiles, DVE
    copies odd tiles (DVE also builds the one-hot rows), so no engine
    exceeds ~23us of work against the ~32us stream.
  - ACT warms Exp+Ln from the combined table set at t~0 via scale=0
    activations (no input dependency), streams 32 exp+accum tiles, and
    issues the final output DMA itself.
  - One-hot H rows and the iota row come from host-side index bookkeeping;
    h lives in a small ring recycled against PE progress.
"""

import math
from contextlib import ExitStack

import numpy as np

import concourse.bacc as bacc
from concourse import mybir
from concourse.bass_utils import run_bass_kernel_spmd

P = 128
C = 722
NCORES = 8
ROWS = 16 * 2048
RPC = ROWS // NCORES  # 4096 rows per core
NT = RPC // P         # 32 row tiles per core
NB = 6
HR = 8                # h ring depth
BLK = [0, 124, 248, 372, 496, 594]  # even bases -> 4B-aligned fp16 slices
URANGES = [(0, 124), (124, 248), (248, 372), (372, 496), (496, 594), (594, 722)]
WDEC = [1.0, math.exp(-0.5), math.exp(-1.0), math.exp(-2.0)]

f32 = mybir.dt.float32
f16 = mybir.dt.float16

AF = mybir.ActivationFunctionType
OP = mybir.AluOpType


def _band_masks() -> np.ndarray:
    """[128, 6*128] f32: block-local band weights, each global band entry
    owned by exactly one block (by min(m,n) ownership range)."""
    m = np.zeros((P, NB * P), np.float32)
    for b in range(NB):
        s = BLK[b]
        lo, hi = URANGES[b]
        for i in range(P):
            for o in range(-3, 4):
                j = i + o
                if 0 <= j < P:
                    mg, ng = s + i, s + j
                    if mg < C and ng < C and lo <= min(mg, ng) < hi:
                        m[i, b * P + j] = WDEC[abs(o)]
    return m


def _combined_act_set(arch: str) -> int:
    """Index of the act-func set containing both Exp and Ln."""
    from concourse.hw_specs import get_activation_tables

    try:
        tables = list(get_activation_tables(arch).items())
    except Exception:
        return -1
    for i, (_, funcs) in enumerate(tables):
        if AF.Exp in funcs and AF.Ln in funcs:
            return i
    return -1


def _build(rpc: int):
    nt = rpc // P
    nc = bacc.Bacc(
        "TRN2", target_bir_lowering=False, debug=False, num_devices=NCORES
    )

    xt = nc.dram_tensor("xt", [P, nt * C], f32, kind="ExternalInput").ap()
    tf = nc.dram_tensor("tf", [P, nt], f32, kind="ExternalInput").ap()
    band = nc.dram_tensor("band", [P, NB * P], f32, kind="ExternalInput").ap()
    iota = nc.dram_tensor("iota16", [P, C], f16, kind="ExternalInput").ap()
    out = nc.dram_tensor("out", [P, nt + 1], f32, kind="ExternalOutput").ap()

    x32 = nc.alloc_sbuf_tensor("x32", [P, nt, C], f32).ap()
    xh = nc.alloc_sbuf_tensor("xh", [P, nt, C], f16).ap()
    h_ring = nc.alloc_sbuf_tensor("h_ring", [P, HR, C], f16).ap()
    esc = nc.alloc_sbuf_tensor("esc", [P, C], f16).ap()
    outsb = nc.alloc_sbuf_tensor("outsb", [P, nt + 1], f32).ap()
    tf_sb = nc.alloc_sbuf_tensor("tf_sb", [P, nt], f32).ap()
    band_sb = nc.alloc_sbuf_tensor("band_sb", [P, NB * P], f32).ap()
    iota_sb = nc.alloc_sbuf_tensor("iota_sb", [P, C], f16).ap()
    warm = nc.alloc_sbuf_tensor("warm", [P, 1], f32).ap()
    warm_o = nc.alloc_sbuf_tensor("warm_o", [P, 1], f32).ap()
    mscr = nc.alloc_sbuf_tensor("mscr", [P, NB * P], f32).ap()

    psum = nc.alloc_psum_tensor("psumblk", [P, NB, 512], f32).ap()

    actset = _combined_act_set(nc.m.arch)

    with (
        nc.Block() as block,
        ExitStack() as _sems,
        nc.semaphore("t_sem") as t_sem,
        nc.semaphore("b_sem") as b_sem,
        nc.semaphore("i_sem") as i_sem,
        nc.semaphore("act_sem") as act_sem,
        nc.semaphore("bnd_sem") as bnd_sem,
        nc.semaphore("vcast") as vcast,
        nc.semaphore("h_sem") as h_sem,
        nc.semaphore("pe_tile") as pe_tile,
        nc.semaphore("odma") as odma,
    ):
        xss = [_sems.enter_context(nc.semaphore(f"xss{i}")) for i in range(nt)]

        @block.sync
        def _(sync):
            for i in range(nt):
                sync.dma_start(
                    out=x32[:, i, :], in_=xt[:, i * C:(i + 1) * C]
                ).then_inc(xss[i], 16)
            sync.wait_ge(odma, 16)

        @block.scalar
        def _(scalar):
            if actset >= 0:
                scalar.add_instruction(
                    mybir.InstLoadActFuncSet(
                        name=nc.get_next_instruction_name(),
                        ins=[],
                        outs=[],
                        act_func_set_id=actset,
                    )
                )
            # scale=0 makes the input irrelevant: warms the tables with no
            # cross-engine dependency
            scalar.activation(out=warm_o, in_=warm, func=AF.Ln,
                              scale=0.0, bias=1.0)
            scalar.activation(out=warm_o, in_=warm, func=AF.Exp,
                              scale=0.0, bias=1.0)
            scalar.dma_start(out=tf_sb, in_=tf).then_inc(t_sem, 16)
            scalar.dma_start(out=iota_sb, in_=iota).then_inc(i_sem, 16)
            scalar.dma_start(out=band_sb, in_=band).then_inc(b_sem, 16)
            for i in range(nt):
                if i >= 2:
                    # act accumulator is pipelined 2-deep
                    scalar.wait_ge(act_sem, i - 1)
                scalar.wait_ge(xss[i], 16)
                scalar.activation(
                    out=esc, in_=x32[:, i, :], func=AF.Exp,
                    accum_out=outsb[:, i:i + 1],
                ).then_inc(act_sem, 1)
            scalar.wait_ge(act_sem, nt)
            scalar.wait_ge(bnd_sem, 1)
            scalar.dma_start(out=out, in_=outsb).then_inc(odma, 16)

        @block.vector
        def _(vector):
            vector.wait_ge(i_sem, 16)
            vector.wait_ge(t_sem, 16)
            for i in range(nt):
                vector.wait_ge(xss[i], 16)
                vector.tensor_copy(
                    out=xh[:, i, :], in_=x32[:, i, :]
                ).then_inc(vcast, 1)
                if i >= HR:
                    vector.wait_ge(pe_tile, i - HR + 1)
                vector.tensor_scalar(
                    out=h_ring[:, i % HR, :], in0=iota_sb,
                    scalar1=tf_sb[:, i:i + 1], scalar2=None,
                    op0=OP.is_equal, op1=OP.bypass,
                ).then_inc(h_sem, 1)
            vector.wait_ge(pe_tile, nt)
            vector.wait_ge(b_sem, 16)
            vector.scalar_tensor_tensor(
                out=mscr.rearrange("p (b n) -> p b n", b=NB),
                in0=psum[:, :, 0:P],
                scalar=0.0,
                in1=band_sb.rearrange("p (b n) -> p b n", b=NB),
                op0=OP.bypass,
                op1=OP.mult,
                accum_out=outsb[:, nt:nt + 1],
            ).then_inc(bnd_sem, 1)

        @block.tensor
        def _(pe):
            for i in range(nt):
                pe.wait_ge(h_sem, i + 1)
                pe.wait_ge(vcast, i + 1)
                for b in range(NB):
                    s = BLK[b]
                    mm = pe.matmul(
                        psum[:, b, 0:P], h_ring[:, i % HR, s:s + P],
                        xh[:, i, s:s + P],
                        start=(i == 0), stop=(i == nt - 1),
                    )
                mm.then_inc(pe_tile, 1)

    nc.compile()
    return nc


def _shard_inputs(prediction: np.ndarray, target: np.ndarray, rpc: int, ncores: int):
    pred = np.asarray(prediction, dtype=np.float32).reshape(-1, C)
    tgt = np.asarray(target).reshape(-1).astype(np.int64)
    nt = rpc // P
    band = _band_masks()
    iota16 = np.broadcast_to(
        np.arange(C, dtype=np.float16)[None, :], (P, C)
    ).copy()
    in_maps = []
    for c in range(ncores):
        sl = slice(c * rpc, (c + 1) * rpc)
        # row i*128+p of the shard -> partition p, tile i
        xtc = np.ascontiguousarray(
            pred[sl].reshape(nt, P, C).transpose(1, 0, 2)
        ).reshape(P, nt * C)
        tfc = np.ascontiguousarray(
            tgt[sl].reshape(nt, P).T.astype(np.float32))
        in_maps.append({
            "xt": xtc,
            "tf": tfc,
            "band": band,
            "iota16": iota16,
        })
    return in_maps


def _host_combine(results, target: np.ndarray, nt: int) -> np.float32:
    tgt = np.asarray(target).reshape(-1).astype(np.int64)
    # W_r = 1 + sum_d w_d*([t>=d] + [t<=C-1-d])
    w_all = np.ones(tgt.shape, np.float64)
    for d in (1, 2, 3):
        w_all += WDEC[d] * (
            (tgt >= d).astype(np.float64) + (tgt <= C - 1 - d).astype(np.float64)
        )
    rpc = len(tgt) // len(results)
    tot = 0.0
    for ci, r in enumerate(results):
        o = np.asarray(r["out"], dtype=np.float64)
        wc = w_all[ci * rpc:(ci + 1) * rpc].reshape(nt, P).T
        tot += (wc * np.log(o[:, :nt])).sum() - o[:, nt:].sum()
    return np.float32(tot / len(tgt))


def kernel(prediction: np.ndarray, target: np.ndarray, _trace: bool = False):
    nc = _build(RPC)
    in_maps = _shard_inputs(prediction, target, RPC, NCORES)
    res = run_bass_kernel_spmd(
        nc, in_maps, core_ids=list(range(NCORES)), trace=_trace
    )
    loss = _host_combine(res.results, target, NT)
    if _trace:
        return loss, res
    return loss
